# revision 21
# baseline (speedup 1.0000x reference)
"""Trainium2 Bass kernel for nn_Ocean_e2e: 48-step advection + 3x3 binomial smoothing.

Pure data-parallel over batch (B=8 -> 8 cores, one 1024x1024 grid each).
State T in fp32(r) block layout T_sb[p, b, w] = X[128*b + p, w]; the tendency
path (a, b coefficient fields, Gx, Gy-copy, P, Q, Z1) runs in bf16 for the
2x DVE mode. All matmuls are f32r (mixing bf16 k=128 matmuls into an f32r
PSUM accumulation group miscomputes on hw). Per step:

  Gy    = Dh @ T + neighbor-block corner matmuls   PE (f32r), PSUM
  gyc   = bf16 copy of Gy                          ACT
  Gx    = x-diff of T -> bf16                      DVE edges + Pool interior
  P     = Gx * a   (in place, bf16 2x)             DVE
  Q     = gyc * b  (in place, bf16 2x)             DVE
  Z1    = Q + P    (in place, bf16 2x)             DVE
  z     = Z1 + T   (f32r)                          DVE/Pool split
  zu/zd = corner rows of z (half DMA gathers)
  C     = Sh @ z + czU/czD k-prefix corner corrs   PE (f32r), PSUM
  c_sb  = C copied out of PSUM                     ACT
  u     = pairwise x-sums of c_sb (1025 wide)      Pool
  Tn    = u + u_shift                              DVE/Pool split

Y-direction boundaries live in the Dh/Sh block weights and corner weights;
x-direction one-sidedness is folded into the host a field; mask==1.
corrU(b) reads zu[0:b] and corrD(b) reads zd[0:b+1] (k prefixes) so each
C block is gated only on mid-stream z blocks, not on z(last).
"""

import numpy as np

DT = 600.0
STEPS = 48
R_EARTH = 6371000.0
DEG2RAD = np.pi / 180.0
B, H, W = 8, 1024, 1024
P = 128
NB = H // P       # 8 row-blocks
N_CORES = 8

_cached = {}
LAST_EXEC_NS = None
Z_DVE = ()
TN_POOL = (7,)
PAIR_PQZ = False
PAIR_LATE = False
PIPE_GX = ()
EDGE_GRP = 2
U_DVE = 0
GY_BUFS = 2
PC_BUFS = 3


# ----------------------------------------------------------------- host math
def _fields(ug, vg, lat, lon, mask):
    """Folded bf16 coefficient fields a, b for one sample."""
    import ml_dtypes
    lat64 = lat.astype(np.float64)
    dlat = float(lat64[1] - lat64[0])
    dlon = float(lon.astype(np.float64)[1] - lon.astype(np.float64)[0])
    dy = R_EARTH * DEG2RAD * dlat
    dx = (R_EARTH * DEG2RAD * dlon) * np.cos(lat64 * DEG2RAD)  # [H]
    xfac = np.full((H, W), 0.5, np.float64) / dx[:, None]
    xfac[:, 0] = 1.0 / dx
    xfac[:, -1] = 1.0 / dx
    yfac = np.full((H, W), 0.5 / dy, np.float64)
    yfac[0, :] = 1.0 / dy
    yfac[-1, :] = 1.0 / dy
    m = mask.astype(np.float64)
    a = (-DT * ug.astype(np.float64) * m * xfac).astype(ml_dtypes.bfloat16)
    b = (-DT * vg.astype(np.float64) * m * yfac).astype(ml_dtypes.bfloat16)
    return a, b


def _block(x):
    """[H, W] -> SBUF block layout [P, NB, W]."""
    return np.ascontiguousarray(x.reshape(NB, P, W).transpose(1, 0, 2))


def _unblock(x):
    """[P, NB, W] -> [H, W]."""
    return np.ascontiguousarray(x.transpose(1, 0, 2).reshape(H, W))


def _matrices():
    """co f32 [128, NB, 512]: dhT 0:128 | shT 128:256 | cup 256:384 | cdn
    384:512 (Gy neighbor-block corner weights). czU/czD [7, NB, 128]: C-stage
    corner weights for the gathered z rows (zu[b-1] -> (block b, m=0)/16,
    zd[b] -> (block b, m=127)/16)."""
    dh = np.zeros((NB, P, P), np.float32)
    sh = np.zeros((NB, P, P), np.float32)
    for bb in range(NB):
        for p in range(P):
            h = bb * P + p
            if h == 0:
                dh[bb, p, p] = -1.0
                dh[bb, p, p + 1] = 1.0
            elif h == H - 1:
                dh[bb, p, p - 1] = -1.0
                dh[bb, p, p] = 1.0
            else:
                if p - 1 >= 0:
                    dh[bb, p, p - 1] = -1.0
                if p + 1 < P:
                    dh[bb, p, p + 1] = 1.0
            sh[bb, p, p] = 2.0 / 16.0
            if p - 1 >= 0:
                sh[bb, p, p - 1] = 1.0 / 16.0
            if p + 1 < P:
                sh[bb, p, p + 1] = 1.0 / 16.0
    dhT = np.ascontiguousarray(dh.transpose(2, 0, 1))  # [k=p, b, m]
    shT = np.ascontiguousarray(sh.transpose(2, 0, 1))

    # Gy corrections as full-k neighbor-block weights:
    # up-corr for block b reads block b-1: row 127 -> out row 0, weight -1
    # dn-corr for block b reads block b+1: row 0 -> out row 127, weight +1
    cup = np.zeros((P, NB, P), np.float32)
    cdn = np.zeros((P, NB, P), np.float32)
    for bb in range(NB):
        if bb > 0:
            cup[127, bb, 0] = -1.0
        if bb < NB - 1:
            cdn[0, bb, 127] = 1.0

    co = np.zeros((P, NB, 512), np.float32)
    co[:, :, 0:128] = dhT
    co[:, :, 128:256] = shT
    co[:, :, 256:384] = cup
    co[:, :, 384:512] = cdn

    # C-stage corner weights on gathered z rows:
    # zu[j] = z[127, j] = Z[128j+127]; used by block j+1 at m=0, /16
    # zd[j] = z[0, j+1] = Z[128(j+1)]; used by block j at m=127, /16
    czU = np.zeros((7, NB, P), np.float32)
    czD = np.zeros((7, NB, P), np.float32)
    for bb in range(NB):
        if bb > 0:
            czU[bb - 1, bb, 0] = 1.0 / 16.0
        if bb < NB - 1:
            czD[bb, bb, P - 1] = 1.0 / 16.0
    return co, czU, czD


# ------------------------------------------------------------- bass program
def build_program(steps=STEPS, unroll=1, debug=False):
    import concourse.mybir as mybir
    import concourse.tile as tile
    from concourse import bacc

    f32 = mybir.dt.float32
    f32r = mybir.dt.float32r
    bf16 = mybir.dt.bfloat16

    assert steps % unroll == 0
    nc = bacc.Bacc("TRN2", target_bir_lowering=False)
    inp_t = nc.dram_tensor("inp_t", [P, NB, W], f32r, kind="ExternalInput")
    inp_ab = nc.dram_tensor("inp_ab", [P, NB, 2 * W], bf16, kind="ExternalInput")
    inp_co = nc.dram_tensor("inp_co", [P, NB, 512], f32r, kind="ExternalInput")
    inp_czU = nc.dram_tensor("inp_czU", [7, NB, P], f32r, kind="ExternalInput")
    inp_czD = nc.dram_tensor("inp_czD", [7, NB, P], f32r, kind="ExternalInput")
    tout = nc.dram_tensor("tout", [P, NB, W], f32r, kind="ExternalOutput")
    if debug:
        dbg_p = nc.dram_tensor("dbg_p", [P, NB, W], bf16, kind="ExternalOutput")
        dbg_q = nc.dram_tensor("dbg_q", [P, NB, W], bf16, kind="ExternalOutput")
        dbg_z = nc.dram_tensor("dbg_z", [P, NB, W], f32r, kind="ExternalOutput")
        dbg_u = nc.dram_tensor("dbg_u", [P, NB, W + 1], f32, kind="ExternalOutput")
        dbg_c = nc.dram_tensor("dbg_c", [P, NB, W], f32r, kind="ExternalOutput")

    HC = W // 2   # 512-wide matmul chunks

    with tile.TileContext(nc) as tc:
        with (
            tc.tile_pool(name="state", bufs=1) as state,
            tc.tile_pool(name="psum_gy", bufs=GY_BUFS, space="PSUM") as pgy,
            tc.tile_pool(name="psum_c", bufs=PC_BUFS, space="PSUM") as pc,
        ):
            t_sb = state.tile([P, NB, W], f32r)
            ab_sb = state.tile([P, NB, 2 * W], bf16)
            co = state.tile([P, NB, 512], f32r)
            czU = state.tile([7, NB, P], f32r)
            czD = state.tile([7, NB, P], f32r)
            p_sb = state.tile([P, NB, W], bf16)      # Gx -> P
            q_sb = state.tile([P, NB, W], bf16)      # gyc -> Q -> Z1
            z_sb = state.tile([P, NB, W], f32r)      # Z = Z1 + T; later C
            u_sb = state.tile([P, NB, W + 1], f32)   # pairwise x-sums
            c_sb = z_sb                              # C staged over z (dead)
            zu = state.tile([7, W], f32r)            # z[127, b], b=0..6
            zd = state.tile([7, W], f32r)            # z[0, b+1], b=0..6

            a_sb = ab_sb[:, :, 0:W]
            b_sb = ab_sb[:, :, W:2 * W]

            # input load split across queues
            for bb in range(NB):
                nc.sync.dma_start(t_sb[:, bb, :], inp_t[:, bb, :])
                nc.gpsimd.dma_start(ab_sb[:, bb, :], inp_ab[:, bb, :])
            nc.scalar.dma_start(co[:], inp_co[:])
            nc.scalar.dma_start(czU[:], inp_czU[:])
            nc.scalar.dma_start(czD[:], inp_czD[:])

            def emit_pqz(g):
                # P = Gx*a, Q = gyc*b, Z1 = Q+P (DVE bf16 2x);
                # z = Z1 + T (f32, DVE/Pool split); then corner gathers
                nc.vector.tensor_mul(
                    p_sb[:, g, :], p_sb[:, g, :], a_sb[:, g, :])
                nc.vector.tensor_mul(
                    q_sb[:, g, :], q_sb[:, g, :], b_sb[:, g, :])
                nc.vector.tensor_add(
                    q_sb[:, g, :], q_sb[:, g, :], p_sb[:, g, :])
                for bb in range(g.start, g.stop):
                    if bb in Z_DVE:
                        nc.vector.tensor_add(
                            z_sb[:, bb, :], q_sb[:, bb, :], t_sb[:, bb, :])
                    else:
                        nc.gpsimd.tensor_add(
                            z_sb[:, bb, :], q_sb[:, bb, :], t_sb[:, bb, :])
                    # z corner-row gathers: first pair sized so C(b0)/C(b1)
                    # unlock right after z(b2) (their Tn gates next step's Gy)
                    if bb == 1:
                        nc.sync.dma_start(zu[0:2, :], z_sb[127:128, 0:2, :])
                    elif bb == 2:
                        nc.sync.dma_start(zd[0:2, :], z_sb[0:1, 1:3, :])
                    elif bb == 3:
                        nc.sync.dma_start(zu[2:4, :], z_sb[127:128, 2:4, :])
                    elif bb == 4:
                        nc.sync.dma_start(zd[2:4, :], z_sb[0:1, 3:5, :])
                    elif bb == 6:
                        nc.sync.dma_start(zu[4:7, :], z_sb[127:128, 4:7, :])
                    elif bb == NB - 1:
                        nc.sync.dma_start(zd[4:7, :], z_sb[0:1, 5:8, :])

            def step(_i):

                # Gx edge cols (disjoint from interior cols - no dep);
                # split so DVE's queue head gates on half the prev-step Tn
                for eb in range(0, NB, EDGE_GRP):
                    e = slice(eb, eb + EDGE_GRP)
                    nc.vector.tensor_sub(
                        p_sb[:, e, 0:W:W - 1],
                        t_sb[:, e, 1:W:W - 2], t_sb[:, e, 0:W - 1:W - 2])

                for bb in range(NB):
                    # Gy = Dh @ T + neighbor-block corner corrections
                    for c in range(2):
                        cs = slice(c * HC, (c + 1) * HC)
                        gy = pgy.tile([P, HC], f32, tag="gy")
                        nc.tensor.matmul(
                            gy[:], co[:, bb, 0:128], t_sb[:, bb, cs],
                            start=True, stop=False)
                        if bb > 0:
                            nc.tensor.matmul(
                                gy[:], co[:, bb, 256:384], t_sb[:, bb - 1, cs],
                                start=False, stop=(bb == NB - 1))
                        if bb < NB - 1:
                            nc.tensor.matmul(
                                gy[:], co[:, bb, 384:512], t_sb[:, bb + 1, cs],
                                start=False, stop=True)
                        nc.scalar.copy(q_sb[:, bb, cs], gy[:])
                    # Gx interior (Pool); with PIPE_GX it was already
                    # computed in the previous step's tail
                    if bb not in PIPE_GX:
                        nc.gpsimd.tensor_sub(
                            p_sb[:, bb, 1:W - 1], t_sb[:, bb, 2:W],
                            t_sb[:, bb, 0:W - 2])
                    if PAIR_LATE and bb in (6, 7):
                        if bb == 7:
                            emit_pqz(slice(6, 8))
                    elif not PAIR_PQZ:
                        emit_pqz(slice(bb, bb + 1))
                    elif bb % 2 == 1:
                        emit_pqz(slice(bb - 1, bb + 1))


                for bb in range(NB):
                    ct = pc.tile([P, W], f32, tag="c")
                    kU = bb               # zu rows 0..bb-1 (needs zu[bb-1])
                    kD = bb + 1           # zd rows 0..bb (needs zd[bb])
                    for c in range(2):
                        cs = slice(c * HC, (c + 1) * HC)
                        nc.tensor.matmul(
                            ct[:, cs], co[:, bb, 128:256], z_sb[:, bb, cs],
                            start=True, stop=False)
                        if bb > 0:
                            nc.tensor.matmul(
                                ct[:, cs], czU[0:kU, bb, :], zu[0:kU, cs],
                                start=False, stop=(bb == NB - 1))
                        if bb < NB - 1:
                            nc.tensor.matmul(
                                ct[:, cs], czD[0:kD, bb, :], zd[0:kD, cs],
                                start=False, stop=True)
                    # u[1..1023] = C[w-1]+C[w]; u[0]=C[0]; u[1024]=C[1023]
                    nc.scalar.copy(
                        u_sb[:, bb, 0:W + 1:W], ct[:, 0:W:W - 1])
                    nc.scalar.copy(c_sb[:, bb, :], ct[:])
                    if bb < U_DVE:
                        nc.vector.tensor_add(
                            u_sb[:, bb, 1:W], c_sb[:, bb, 0:W - 1],
                            c_sb[:, bb, 1:W])
                    else:
                        nc.gpsimd.tensor_add(
                            u_sb[:, bb, 1:W], c_sb[:, bb, 0:W - 1],
                            c_sb[:, bb, 1:W])
                    # Tn = u + u_shift (f32), split DVE/Pool
                    if bb not in TN_POOL:
                        nc.vector.tensor_add(
                            t_sb[:, bb, :], u_sb[:, bb, 0:W], u_sb[:, bb, 1:W + 1])
                    else:
                        nc.gpsimd.tensor_add(
                            t_sb[:, bb, :], u_sb[:, bb, 0:W], u_sb[:, bb, 1:W + 1])
                    # software-pipelined Gx for the NEXT step (reads the Tn
                    # just written; lands in Pool's queue before next step's
                    # u/Tn tail so P(b) is never seam-stalled)
                    if bb in PIPE_GX:
                        nc.gpsimd.tensor_sub(
                            p_sb[:, bb, 1:W - 1], t_sb[:, bb, 2:W],
                            t_sb[:, bb, 0:W - 2])

            if PIPE_GX:
                for bb in PIPE_GX:
                    nc.gpsimd.tensor_sub(
                        p_sb[:, bb, 1:W - 1], t_sb[:, bb, 2:W],
                        t_sb[:, bb, 0:W - 2])

            with tc.For_i(0, steps // unroll,
                          hint_engines=(mybir.EngineType.PE,)) as _i:
                for _u in range(unroll):
                    step(_i)

            for bb in range(NB):
                eng = (nc.sync, nc.gpsimd)[bb % 2]
                eng.dma_start(tout[:, bb, :], t_sb[:, bb, :])

            if debug:
                nc.sync.dma_start(dbg_p[:], p_sb[:])
                nc.sync.dma_start(dbg_q[:], q_sb[:])
                nc.sync.dma_start(dbg_z[:], z_sb[:])
                nc.sync.dma_start(dbg_u[:], u_sb[:])
                nc.sync.dma_start(dbg_c[:], z_sb[:])

    nc.finalize()
    return nc


# ------------------------------------------------------------------- driver
def kernel(T, ug, vg, lat, lon, mask):
    import ml_dtypes
    from concourse import bass_utils

    key = STEPS
    if key not in _cached:
        _cached[key] = build_program(STEPS, unroll=48)
    nc = _cached[key]

    co, czU, czD = _matrices()
    in_maps = []
    for s in range(B):
        a, bfld = _fields(ug[s], vg[s], lat, lon, mask)
        ab = np.concatenate(
            [_block(a), _block(bfld)], axis=2).astype(ml_dtypes.bfloat16)
        in_maps.append({
            "inp_t": _block(T[s].astype(np.float32)),
            "inp_ab": ab,
            "inp_co": co,
            "inp_czU": czU,
            "inp_czD": czD,
        })

    res = bass_utils.run_bass_kernel_spmd(nc, in_maps, core_ids=list(range(N_CORES)))
    global LAST_EXEC_NS
    if res.exec_time_ns is not None:
        LAST_EXEC_NS = res.exec_time_ns
    out = np.stack([_unblock(r["tout"]) for r in res.results])
    return (out * mask[None].astype(np.float32)).astype(T.dtype)

